# revision 24
# baseline (speedup 1.0000x reference)
"""Trainium2 Bass kernel for an AttentionBlock (GroupNorm + 1x1-conv QKV +
4-head attention over 48x48 pixels + 1x1-conv proj + residual).

Contract: kernel(**inputs) takes the FULL unsharded inputs (as produced by
setup_inputs) and returns the FULL output (8, 256, 48, 48) float32.

Strategy: data-parallel over batch - batch element i runs on NeuronCore i.
All parameters are replicated. Per core everything stays on-chip:

  x [256, 2304] (channels on partitions), DMA'd in 768-col chunks so
  GroupNorm stats (bn_stats at 384-col grain) trail the load; ScalarE
  activation tables (Identity/Sqrt/Exp) are preloaded under the DMA
  -> GroupNorm via bn_stats + per-tile block-diagonal gsel matmul -> A,B
  -> xn produced per 512-col chunk, alternating DVE / ScalarE so the first
     q,k matmuls start as soon as chunk 0 exists
  -> q,k = W_qk @ xn  (channels-on-partitions, bf16; q pre-scaled 1/8); the
     two heads of a pair live in rows 0-63 / 64-127 of one 128-row tile
  -> vT = xn^T @ W_v in fp8e4m3, stored as [128, 2, 512] pb-pair tiles with a
     per-head ones column (softmax denominators ride the PV matmul for free)
  -> scores: the two heads of a pair issued as K=64 matmuls on DISTINCT PE
     row strips (base partitions 0 / 64) so the systolic array runs them
     concurrently (row tiling)
  -> softmax exp WITHOUT max subtraction (scores are provably small): split
     across ScalarE (exact exp -> fp8) and VectorE (Schraudolph bit-trick:
     E = int8(s*8/ln2 + C) bitcast as fp8e4m3); normalization makes both
     accurate to ~3e-4 end-to-end
  -> PV in fp8 DoubleRow perf mode: two 128-row j-blocks per matmul
     (contraction 256), unnormalized, denominators in PSUM rows 64/96;
     PV trails exp by LA=3 pb-pair groups
  -> per-irange epilogue, fully on-chip (no DRAM round trip):
     ScalarE Identity copies the PSUM denominator rows to SBUF row 0 (the
     custom-DVE reciprocal misreads PSUM at base 64/96 and silently writes
     zeros at base 64 on hardware, so everything sits at partition base 0),
     reciprocal_approx_fast on DVE, two tiny SBUF->SBUF DMAs pack the rows
     adjacent, then a K=2 bsel matmul broadcasts 1/den over each head's 64
     rows and one [128,il] multiply normalizes.  The tensor-engine part is
     deferred into the NEXT irange (pbp==3) so the PE queue never stalls
     on the reciprocal chain
  -> pair 1 additionally streams proj + residual + output DMA per pixel
     chunk inside the same deferred epilogue (residual base x + eff-proj-
     bias precomputed on ScalarE; v-bias folded into the proj bias on host)
"""

from contextlib import ExitStack

import numpy as np

import concourse.bacc as bacc
import concourse.bass as bass
import concourse.mybir as mybir
import concourse.tile as tile
from concourse.bass_utils import run_bass_kernel_spmd

F32 = mybir.dt.float32
F32R = mybir.dt.float32r
BF16 = mybir.dt.bfloat16
FP8 = mybir.dt.float8e4
I8 = mybir.dt.int8
AF = mybir.ActivationFunctionType
OP = mybir.AluOpType

N_CORES = 8
C = 256          # channels
HW = 2304        # 48*48 pixels
NH = 4           # heads
HD = 64          # head dim
G = 32           # groupnorm groups
EPS = 1e-5
CT = 2           # channel partition tiles of 128
PB = 18          # pixel blocks of 128
PBP = 9          # pixel block pairs (DoubleRow processes 2 j-blocks/matmul)

# pixel chunks for N<=512 matmuls
PCH = [(0, 512), (512, 512), (1024, 512), (1536, 512), (2048, 256)]
# x DMA chunks (768 wide: fewer DMA issues, aligned to 256-col bn subgroups)
XCH = [(0, 768), (768, 768), (1536, 768)]
# i-ranges for the attention stage: 512-wide so a pb-PAIR's scores fit one
# [128, 1024] PSUM tile (2 banks) and ONE exp instruction covers both
IRANGES = [(0, 512), (512, 512), (1024, 512), (1536, 512), (2048, 256)]

# Schraudolph exp->fp8e4m3 constants: bits = trunc(s * 8/ln2 + C8)
A8 = 11.541560327111707
C8 = 55.75

# ones column (within each head's 128-col lhsT block) = the PSUM partition
# the softmax denominator lands on; {64, 96} are legal engine AP bases.
ONES_COL = {0: 64, 1: 96}  # by head parity

LA = 3  # PV lookahead in pb-pairs


# exp engine pattern per (group, head): 'S'=ScalarE exact exp, 'V'=DVE
# Schraudolph. 6/10 on ScalarE; 4/5 groups split both engines so the two
# heads' chains advance in lockstep (no stagger).
# 9-entry pattern aligned with DVE's epilogue load: both-on-ScalarE groups
# sit at pbp 0 (DVE runs the previous irange's reciprocals) and pbp 4 (DVE
# runs the deferred normalize multiply + residual adds)
EXP_PAT = [
    ("S", "S"), ("S", "V"), ("V", "S"), ("S", "V"), ("S", "S"),
    ("V", "S"), ("S", "V"), ("V", "S"), ("S", "V"),
]


def _build():
    nc = bacc.Bacc(
        "TRN2", target_bir_lowering=False, debug=False, num_devices=N_CORES
    )
    x_d = nc.dram_tensor("x", [C, HW], F32, kind="ExternalInput")
    wqkvT_d = nc.dram_tensor("wqkvT", [C, 3 * C], F32R, kind="ExternalInput")
    wprojT_d = nc.dram_tensor("wprojT", [C, C], BF16, kind="ExternalInput")
    gsel_d = nc.dram_tensor("gsel", [C, C], F32, kind="ExternalInput")
    # per-channel vectors: [...,0]=gn_w [...,1]=gn_b [...,2]=qb/8 [...,3]=kb
    # [...,4]=proj_b + proj_w @ v_bias
    vecs_d = nc.dram_tensor("vecs", [CT, 128, 5], F32, kind="ExternalInput")
    vmask_d = nc.dram_tensor("vmask", [128, 2, 4 * 128], FP8, kind="ExternalInput")
    bsel_d = nc.dram_tensor("bsel", [128, 128], F32R, kind="ExternalInput")
    out_d = nc.dram_tensor("out", [C, HW], F32, kind="ExternalOutput")

    with ExitStack() as ctx:
        tc = ctx.enter_context(tile.TileContext(nc))
        const = ctx.enter_context(tc.tile_pool(name="const", bufs=1))
        big = ctx.enter_context(tc.tile_pool(name="big", bufs=1))
        xin = ctx.enter_context(tc.tile_pool(name="xin", bufs=1))
        xno = ctx.enter_context(tc.tile_pool(name="xno", bufs=1))
        epool = ctx.enter_context(tc.tile_pool(name="epool", bufs=10))
        small = ctx.enter_context(tc.tile_pool(name="small", bufs=1))
        mmps = ctx.enter_context(
            tc.tile_pool(name="mmps", bufs=2, space=bass.MemorySpace.PSUM)
        )
        pvps = ctx.enter_context(
            tc.tile_pool(name="pvps", bufs=2, space=bass.MemorySpace.PSUM)
        )

        # ---- load inputs ----
        # x in PCH column chunks per 128-row tile so GN stats trail the DMA
        vecs_sb = []
        wqkvT_sb = []
        wprojT_sb = []
        gsel_sb = []
        x_sb = [[None] * len(XCH) for _ in range(CT)]
        # x chunk-major first: GroupNorm stats trail this stream directly
        for cx, (po, pl) in enumerate(XCH):
            for ct in range(CT):
                xt = xin.tile([128, pl], F32, tag=f"x{ct}_{cx}", name=f"x{ct}_{cx}")
                nc.sync.dma_start(xt[:], x_d[ct * 128 : (ct + 1) * 128, po : po + pl])
                x_sb[ct][cx] = xt
        for ct in range(CT):
            vt = const.tile([128, 5], F32, tag=f"vecs{ct}")
            nc.sync.dma_start(vt[:], vecs_d[ct])
            vecs_sb.append(vt)
            gs = const.tile([128, C], F32, tag=f"gsel{ct}")
            nc.sync.dma_start(gs[:], gsel_d[ct * 128 : (ct + 1) * 128, :])
            gsel_sb.append(gs)
        for ct in range(CT):
            wq = const.tile([128, 3 * C], F32R, tag=f"wqkv{ct}")
            nc.sync.dma_start(wq[:], wqkvT_d[ct * 128 : (ct + 1) * 128, :])
            wqkvT_sb.append(wq)
        for ct in range(CT):
            wp = const.tile([128, C], BF16, tag=f"wproj{ct}")
            nc.sync.dma_start(wp[:], wprojT_d[ct * 128 : (ct + 1) * 128, :])
            wprojT_sb.append(wp)
        vmask_sb = const.tile([128, 2, 4 * 128], FP8, tag="vmask")
        nc.sync.dma_start(vmask_sb[:], vmask_d[:])
        # bsel rows {0,1} broadcast the two 1/denominator rows over each
        # head's 64 rows within a pair's 128-row block (K=2 matmul)
        bsel_sb = const.tile([128, 128], F32R, tag="bsel")
        nc.sync.dma_start(bsel_sb[:], bsel_d[:])

        # ---- ScalarE activation-table preload (dummy ops): the Exp and
        # Sqrt table loads cost ~1.3us each; issue them under the input DMA
        # instead of on the critical path ----
        tldr = small.tile([128, 1], F32, tag="tldr")
        nc.vector.memset(tldr[:], 1.0)
        nc.scalar.activation(tldr[:], tldr[:], AF.Identity)
        nc.scalar.activation(tldr[:], tldr[:], AF.Sqrt)
        nc.scalar.activation(tldr[:], tldr[:], AF.Exp)

        # ---- GroupNorm statistics ----
        # per-channel mean/var via bn_stats (9 subgroups of 256, chunked with
        # the x DMA), then average groups of 8 channels with the gsel matmul
        # on [mean, E[x^2]].  Emission is chunk-major so both channel tiles'
        # stats trail the DMA stream.
        st_sb = [
            small.tile([128, 6, 6], F32, tag=f"bnst{ct}", name=f"bnst{ct}")
            for ct in range(CT)
        ]
        for cx, (po, pl) in enumerate(XCH):
            for ct in range(CT):
                xr = x_sb[ct][cx].rearrange("p (n f) -> p n f", f=384)
                for k in range(2):
                    nc.vector.bn_stats(st_sb[ct][:, 2 * cx + k, :], xr[:, k, :])
        stats2 = []
        for ct in range(CT):
            mv = small.tile([128, 2], F32, tag=f"mv{ct}")
            nc.vector.bn_aggr(mv[:], st_sb[ct][:])
            s2 = small.tile([128, 2], F32, tag=f"s2{ct}")
            nc.vector.tensor_copy(s2[:, 0:1], mv[:, 0:1])
            nc.vector.tensor_tensor(s2[:, 1:2], mv[:, 0:1], mv[:, 0:1], op=OP.mult)
            nc.vector.tensor_add(s2[:, 1:2], s2[:, 1:2], mv[:, 1:2])
            stats2.append(s2)

        eps_sb = small.tile([128, 1], F32, tag="eps")
        nc.vector.memset(eps_sb[:], EPS)
        # 1/denominator: reciprocal lands in rscr rows {0,64} (legal DVE
        # write bases), then two tiny SBUF->SBUF DMAs pack them into rsum
        # rows {0,1} for the K=2 broadcast matmul (rows reused across pairs)
        rsum = small.tile([128, HW], F32R, tag="rsum")
        rscr = small.tile([128, 1024], F32, tag="rscr")
        dens = small.tile([128, 1024], F32, tag="dens")
        A_sb = []
        B_sb = []
        for mb in range(CT):
            # gsel is block-diagonal in 8-channel groups, so channel tile mb
            # only needs its own stats — no cross-tile accumulation
            ps = mmps.tile([128, 2], F32, tag=("stA" if mb % 2 == 0 else "stB"), bufs=1)
            nc.tensor.matmul(
                ps[:],
                gsel_sb[mb][:, mb * 128 : (mb + 1) * 128],
                stats2[mb][:],
                start=True,
                stop=True,
            )
            rstd = small.tile([128, 1], F32, tag=f"rstd{mb}")
            msq = small.tile([128, 1], F32, tag=f"msq{mb}")
            mg = small.tile([128, 1], F32, tag=f"mg{mb}")
            nc.vector.tensor_copy(mg[:], ps[:, 0:1])
            nc.vector.tensor_tensor(msq[:], mg[:], mg[:], op=OP.mult)
            nc.vector.tensor_tensor(rstd[:], ps[:, 1:2], msq[:], op=OP.subtract)
            # rstd = 1/sqrt(var + eps)
            nc.scalar.activation(rstd[:], rstd[:], AF.Sqrt, bias=eps_sb[:])
            nc.vector.reciprocal(rstd[:], rstd[:])
            a = small.tile([128, 1], F32, tag=f"A{mb}")
            b = small.tile([128, 1], F32, tag=f"B{mb}")
            nc.vector.tensor_tensor(a[:], vecs_sb[mb][:, 0:1], rstd[:], op=OP.mult)
            nc.vector.tensor_tensor(b[:], mg[:], a[:], op=OP.mult)
            nc.vector.tensor_tensor(b[:], vecs_sb[mb][:, 1:2], b[:], op=OP.subtract)
            A_sb.append(a)
            B_sb.append(b)

        # xn per (ct, chunk); residual base ot = x + eff_proj_bias on ScalarE
        # (idle at this point) per (ct, chunk)
        def x_cols(ct, lo, ln):
            "source APs for x columns [lo, lo+ln) out of the 768-wide chunks"
            out = []
            while ln > 0:
                cx = lo // 768
                off = lo - 768 * cx
                take = min(ln, 768 - off)
                out.append((x_sb[ct][cx][:, off : off + take], lo, take))
                lo += take
                ln -= take
            return out

        xn_sb = [[None] * len(PCH) for _ in range(CT)]
        for ci in range(len(PCH)):
            po, pl = PCH[ci]
            for ct in range(CT):
                xn = xno.tile([128, pl], F32R, tag=f"xn{ct}_{ci}", name=f"xn{ct}_{ci}")
                for src_ap, lo, take in x_cols(ct, po, pl):
                    dst = xn[:, lo - po : lo - po + take]
                    if (2 * ci + ct) % 2 == 0:
                        nc.vector.tensor_scalar(
                            dst, src_ap, A_sb[ct][:], B_sb[ct][:],
                            op0=OP.mult, op1=OP.add,
                        )
                    else:
                        nc.scalar.activation(
                            dst, src_ap, AF.Identity,
                            bias=B_sb[ct][:], scale=A_sb[ct][:],
                        )
                xn_sb[ct][ci] = xn
        # residual base ot = x + eff_proj_bias, one tile per ct (consumed only
        # in pair 1, so coarse write tracking is harmless)
        ot_sb = []
        for ct in range(CT):
            ot = xno.tile([128, HW], F32, tag=f"ot{ct}", name=f"ot{ct}")
            for cx, (po, pl) in enumerate(XCH):
                nc.scalar.activation(
                    ot[:, po : po + pl], x_sb[ct][cx][:], AF.Identity,
                    bias=vecs_sb[ct][:, 4:5],
                )
            ot_sb.append(ot)

        def xn_cols(ct, lo, ln):
            """AP for xn columns [lo, lo+ln) — must lie within one chunk."""
            ci = lo // 512
            return xn_sb[ct][ci][:, lo - 512 * ci : lo - 512 * ci + ln]

        # ---- qkv: q,k bf16 in channel-layout [128, HW] per head PAIR ----
        # chunk-major so the first matmuls start as soon as xn chunk 0 exists
        q_sb = [
            big.tile([128, HW], BF16, tag=f"q{p}", name=f"q{p}") for p in range(CT)
        ]
        k_sb = [
            big.tile([128, HW], BF16, tag=f"k{p}", name=f"k{p}") for p in range(CT)
        ]
        for ip, (po, pl) in enumerate(PCH):
            for which in range(2):  # 0 -> q, 1 -> k
                woff = which * C
                for mb in range(CT):
                    ps = mmps.tile(
                        [128, 1024], F32,
                        tag=("stA" if (ip * 2 + which) % 2 == 0 else "stB"), bufs=1,
                    )
                    for kt in range(CT):
                        nc.tensor.matmul(
                            ps[:, :pl],
                            wqkvT_sb[kt][
                                :, woff + mb * 128 : woff + (mb + 1) * 128
                            ],
                            xn_sb[kt][ip][:],
                            start=(kt == 0),
                            stop=(kt == CT - 1),
                        )
                    if which == 0:
                        # q = (q_raw + qb) / 8  (qb/8 is precomputed on host)
                        nc.vector.tensor_scalar(
                            q_sb[mb][:, po : po + pl],
                            ps[:, :pl],
                            0.125,
                            vecs_sb[mb][:, 2:3],
                            op0=OP.mult,
                            op1=OP.add,
                        )
                    else:
                        nc.vector.tensor_scalar(
                            k_sb[mb][:, po : po + pl],
                            ps[:, :pl],
                            vecs_sb[mb][:, 3:4],
                            None,
                            op0=OP.add,
                        )

        # ---- attention: one head PAIR at a time, pb-PAIR groups. The two
        # heads' K=64 score matmuls go to distinct PE row strips (base
        # partitions 0/64); a group's two score matmuls share one [128, 1024]
        # PSUM tile so ONE exp instruction covers both pb of the group.
        # vT tiles (fp8, with per-head ones column from vmask) are produced
        # just-in-time during pair 0's first range. ----
        attn_sb = [
            big.tile([128, HW], BF16, tag=f"attn{p}", name=f"attn{p}")
            for p in range(CT)
        ]
        vt2 = []
        _tagc = [0]

        def next_tag():
            _tagc[0] += 1
            return ("stA", "stB", "stC")[_tagc[0] % 3]

        def produce_vt2(pbp):
            vt = big.tile([128, 2, 4 * 128], FP8, tag=f"vt{pbp}", name=f"vt{pbp}")
            nc.sync.dma_start(vt[:], vmask_sb[:])
            for par in range(2):
                pb = 2 * pbp + par
                ps = mmps.tile([128, 1024], F32, tag=next_tag(), bufs=1, name="vps")
                for kt in range(CT):
                    nc.tensor.matmul(
                        ps[:, :C],
                        xn_cols(kt, pb * 128, 128),
                        wqkvT_sb[kt][:, 2 * C : 3 * C],
                        start=(kt == 0),
                        stop=(kt == CT - 1),
                    )
                dst = vt[:, par, :].rearrange("p (h c) -> p h c", h=NH)[:, :, 0:HD]
                src = ps[:, :C].rearrange("p (h c) -> p h c", c=HD)
                nc.vector.tensor_copy(dst, src)
            vt2.append(vt)

        def emit_epilogue(p, io, il, pvs):
            """Broadcast 1/denominator over each head's 64 rows (two
            concurrent K=1 matmuls on disjoint PE tiles), normalize the attn
            chunk, and for pair 1 stream proj + residual + output DMA.
            Deferred past the next irange's first score groups so the PE
            queue never stalls waiting on the DVE reciprocal."""
            rs = mmps.tile([128, 1024], F32, tag=next_tag(), bufs=1, name="rs")
            nc.tensor.matmul(
                rs[:, :il],
                bsel_sb[0:2, :],
                rsum[0:2, io : io + il],
                start=True,
                stop=True,
            )
            nc.vector.tensor_tensor(
                attn_sb[p][:, io : io + il],
                attn_sb[p][:, io : io + il],
                rs[:, :il],
                op=OP.mult,
            )
            if p == CT - 1:
                for ct in range(CT):
                    ps = mmps.tile(
                        [128, 1024], F32, tag=next_tag(), bufs=1, name="prj"
                    )
                    for kt in range(CT):
                        nc.tensor.matmul(
                            ps[:, :il],
                            wprojT_sb[kt][:, ct * 128 : (ct + 1) * 128],
                            attn_sb[kt][:, io : io + il],
                            start=(kt == 0),
                            stop=(kt == CT - 1),
                        )
                    nc.vector.tensor_tensor(
                        ot_sb[ct][:, io : io + il],
                        ot_sb[ct][:, io : io + il],
                        ps[:, :il],
                        op=OP.add,
                    )
                    nc.sync.dma_start(
                        out_d[ct * 128 : (ct + 1) * 128, io : io + il],
                        ot_sb[ct][:, io : io + il],
                    )

        epi_pending = []
        for p in range(CT):
            for ri, (io, il) in enumerate(IRANGES):
                es = {}
                pvs = {}
                for hh in range(2):
                    pvs[hh] = pvps.tile(
                        [128, 512], F32, tag="pvps", name=f"pv{p}_{io}_{hh}"
                    )

                def emit_pv(pbp, p=p, io=io, il=il, es=es, pvs=pvs):
                    for hh in range(2):
                        h = 2 * p + hh
                        nc.tensor.matmul(
                            pvs[hh][:, :il],
                            vt2[pbp][:, :, h * 128 : (h + 1) * 128],
                            es[(hh, pbp)][:, :, :il],
                            start=(pbp == 0),
                            stop=(pbp == PBP - 1),
                            perf_mode=mybir.MatmulPerfMode.DoubleRow,
                        )

                for pbp in range(PBP):
                    if p == 0 and ri == 0:
                        produce_vt2(pbp)
                    pat = EXP_PAT[pbp]
                    sts = {}
                    for hh in range(2):
                        sts[hh] = mmps.tile(
                            [128, 1024], F32, tag=next_tag(), bufs=1,
                            name=f"st{hh}",
                        )
                    for par in range(2):
                        pb = 2 * pbp + par
                        for hh in range(2):
                            ro = hh * 64
                            nc.tensor.matmul(
                                sts[hh][:, par * il : par * il + il],
                                k_sb[p][ro : ro + 64, pb * 128 : (pb + 1) * 128],
                                q_sb[p][ro : ro + 64, io : io + il],
                                start=True,
                                stop=True,
                            )
                    for hh in range(2):
                        es[(hh, pbp)] = epool.tile(
                            [128, 2, il], FP8, tag="E",
                            name=f"e{p}_{io}_{hh}_{pbp}",
                        )
                        e = es[(hh, pbp)]
                        src = sts[hh][:, : 2 * il].rearrange(
                            "p (a b) -> p a b", a=2
                        )
                        if pat[hh] == "V":
                            nc.vector.tensor_scalar(
                                e[:].bitcast(I8),
                                src,
                                A8,
                                C8,
                                op0=OP.mult,
                                op1=OP.add,
                            )
                        else:
                            nc.scalar.activation(e[:], src, AF.Exp)
                    if pbp == 3 and epi_pending:
                        emit_epilogue(*epi_pending.pop())
                    if pbp - LA >= 0:
                        emit_pv(pbp - LA)
                for pbp in range(PBP - LA, PBP):
                    emit_pv(pbp)

                # attn rows out of PSUM (ScalarE) + denominator reciprocals
                # (DVE) issue immediately; the tensor-engine part of the
                # epilogue is deferred into the next irange
                for hh in range(2):
                    srow = ONES_COL[hh]
                    co = 512 * hh
                    # Identity (not Copy) so the ScalarE table set during
                    # attention stays {Exp, Identity} -> no table reloads
                    nc.scalar.activation(
                        attn_sb[p][hh * 64 : (hh + 1) * 64, io : io + il],
                        pvs[hh][0:64, :il],
                        AF.Identity,
                    )
                    # PSUM denominator row -> SBUF row 0 on ScalarE, then
                    # reciprocal SBUF->SBUF on DVE.  Both at partition base 0:
                    # the custom-DVE op silently writes zeros at base 64 and
                    # misreads PSUM at base 64/96 on hardware.
                    nc.scalar.activation(
                        dens[0:1, co : co + il],
                        pvs[hh][srow : srow + 1, :il],
                        AF.Identity,
                    )
                    nc.vector.reciprocal_approx_fast(
                        rscr[0:1, co : co + il],
                        dens[0:1, co : co + il],
                    )
                for hh in range(2):
                    nc.sync.dma_start(
                        rsum[hh : hh + 1, io : io + il],
                        rscr[0:1, 512 * hh : 512 * hh + il].bitcast(F32R),
                    )
                epi_pending.append((p, io, il, pvs))

        while epi_pending:
            emit_epilogue(*epi_pending.pop())

    nc.compile()
    return nc


_NC = None


def _get_nc():
    global _NC
    if _NC is None:
        _NC = _build()
    return _NC


def _host_prep(x, gn_w, gn_b, qkv_w, qkv_b, proj_w, proj_b):
    import ml_dtypes

    f32 = np.float32
    x = np.asarray(x, dtype=f32)
    gn_w = np.asarray(gn_w, dtype=f32)
    gn_b = np.asarray(gn_b, dtype=f32)
    qkv_w = np.asarray(qkv_w, dtype=f32)
    qkv_b = np.asarray(qkv_b, dtype=f32)
    proj_w = np.asarray(proj_w, dtype=f32)
    proj_b = np.asarray(proj_b, dtype=f32)

    b = x.shape[0]
    xs = np.ascontiguousarray(x.reshape(b, C, HW))

    wqkvT = np.ascontiguousarray(qkv_w.T)
    wprojT = np.ascontiguousarray(proj_w.T).astype(ml_dtypes.bfloat16)

    gsel = np.zeros((C, C), dtype=f32)
    for g in range(G):
        gsel[g * 8 : (g + 1) * 8, g * 8 : (g + 1) * 8] = 1.0 / 8.0

    bsel = np.zeros((128, 128), dtype=f32)
    bsel[0, 0:64] = 1.0
    bsel[1, 64:128] = 1.0

    pbeff = proj_b + proj_w @ qkv_b[2 * C : 3 * C]
    vecs = np.stack(
        [gn_w, gn_b, qkv_b[:C] / 8.0, qkv_b[C : 2 * C], pbeff], axis=-1
    ).reshape(CT, 128, 5)
    vecs = np.ascontiguousarray(vecs.astype(f32))

    vmask = np.zeros((128, 2, 4 * 128), dtype=np.float32)
    for h in range(NH):
        vmask[:, :, h * 128 + ONES_COL[h % 2]] = 1.0
    vmask = vmask.astype(ml_dtypes.float8_e4m3)

    shared = {
        "wqkvT": wqkvT,
        "wprojT": wprojT,
        "gsel": gsel,
        "bsel": bsel,
        "vecs": vecs,
        "vmask": vmask,
    }
    in_maps = [dict(shared, x=np.ascontiguousarray(xs[i])) for i in range(b)]
    return in_maps, x.shape


def _run(inputs, **run_kwargs):
    nc = _get_nc()
    in_maps, xshape = _host_prep(**inputs)
    res = run_bass_kernel_spmd(
        nc, in_maps, core_ids=list(range(N_CORES)), **run_kwargs
    )
    out = np.stack([res.results[i]["out"] for i in range(N_CORES)])
    return out.reshape(xshape).astype(np.float32), res


def kernel(**inputs):
    out, _ = _run(inputs)
    return out


# revision 25
# speedup vs baseline: 1.0360x; 1.0360x over previous
"""Trainium2 Bass kernel for an AttentionBlock (GroupNorm + 1x1-conv QKV +
4-head attention over 48x48 pixels + 1x1-conv proj + residual).

Contract: kernel(**inputs) takes the FULL unsharded inputs (as produced by
setup_inputs) and returns the FULL output (8, 256, 48, 48) float32.

Strategy: data-parallel over batch - batch element i runs on NeuronCore i.
All parameters are replicated. Per core everything stays on-chip:

  x [256, 2304] (channels on partitions), DMA'd in 768-col chunks so
  GroupNorm stats (bn_stats at 384-col grain) trail the load; ScalarE
  activation tables (Identity/Sqrt/Exp) are preloaded under the DMA
  -> GroupNorm via bn_stats + per-tile block-diagonal gsel matmul -> A,B
  -> xn produced per 512-col chunk, alternating DVE / ScalarE so the first
     q,k matmuls start as soon as chunk 0 exists
  -> q,k = W_qk @ xn  (channels-on-partitions, bf16; q pre-scaled 1/8); the
     two heads of a pair live in rows 0-63 / 64-127 of one 128-row tile
  -> vT = xn^T @ W_v in fp8e4m3, stored as [128, 2, 512] pb-pair tiles with a
     per-head ones column (softmax denominators ride the PV matmul for free)
  -> scores: the two heads of a pair issued as K=64 matmuls on DISTINCT PE
     row strips (base partitions 0 / 64) so the systolic array runs them
     concurrently (row tiling)
  -> softmax exp WITHOUT max subtraction (scores are provably small): split
     across ScalarE (exact exp -> fp8) and VectorE (Schraudolph bit-trick:
     E = int8(s*8/ln2 + C) bitcast as fp8e4m3); normalization makes both
     accurate to ~3e-4 end-to-end
  -> PV in fp8 DoubleRow perf mode: two 128-row j-blocks per matmul
     (contraction 256), unnormalized, denominators in PSUM rows 64/96;
     PV trails exp by LA=3 pb-pair groups
  -> per-irange epilogue, fully on-chip (no DRAM round trip):
     ScalarE Identity copies the PSUM denominator rows to SBUF row 0 (the
     custom-DVE reciprocal misreads PSUM at base 64/96 and silently writes
     zeros at base 64 on hardware, so everything sits at partition base 0),
     reciprocal_approx_fast on DVE, two tiny SBUF->SBUF DMAs pack the rows
     adjacent, then a K=2 bsel matmul broadcasts 1/den over each head's 64
     rows and one [128,il] multiply normalizes.  The tensor-engine part is
     deferred into the NEXT irange (pbp==3) so the PE queue never stalls
     on the reciprocal chain
  -> pair 1 additionally streams proj + residual + output DMA per pixel
     chunk inside the same deferred epilogue (residual base x + eff-proj-
     bias precomputed on ScalarE; v-bias folded into the proj bias on host)
"""

from contextlib import ExitStack

import numpy as np

import concourse.bacc as bacc
import concourse.bass as bass
import concourse.mybir as mybir
import concourse.tile as tile
from concourse.bass_utils import run_bass_kernel_spmd

F32 = mybir.dt.float32
F32R = mybir.dt.float32r
BF16 = mybir.dt.bfloat16
FP8 = mybir.dt.float8e4
I8 = mybir.dt.int8
AF = mybir.ActivationFunctionType
OP = mybir.AluOpType

N_CORES = 8
C = 256          # channels
HW = 2304        # 48*48 pixels
NH = 4           # heads
HD = 64          # head dim
G = 32           # groupnorm groups
EPS = 1e-5
CT = 2           # channel partition tiles of 128
PB = 18          # pixel blocks of 128
PBP = 9          # pixel block pairs (DoubleRow processes 2 j-blocks/matmul)

# pixel chunks for N<=512 matmuls
PCH = [(0, 512), (512, 512), (1024, 512), (1536, 512), (2048, 256)]
# x DMA chunks (768 wide: fewer DMA issues, aligned to 256-col bn subgroups)
XCH = [(0, 768), (768, 768), (1536, 768)]
# i-ranges for the attention stage: 512-wide so a pb-PAIR's scores fit one
# [128, 1024] PSUM tile (2 banks) and ONE exp instruction covers both
IRANGES = [(0, 512), (512, 512), (1024, 512), (1536, 512), (2048, 256)]

# Schraudolph exp->fp8e4m3 constants: bits = trunc(s * 8/ln2 + C8)
A8 = 11.541560327111707
C8 = 55.75

# ones column (within each head's 128-col lhsT block) = the PSUM partition
# the softmax denominator lands on; {64, 96} are legal engine AP bases.
ONES_COL = {0: 64, 1: 96}  # by head parity

LA = 3  # PV lookahead in pb-pairs


# exp engine pattern per (group, head): 'S'=ScalarE exact exp, 'V'=DVE
# Schraudolph. 6/10 on ScalarE; 4/5 groups split both engines so the two
# heads' chains advance in lockstep (no stagger).
EXP_PAT = [("S", "V"), ("V", "S"), ("S", "V"), ("V", "S"), ("S", "S")]


def _build():
    nc = bacc.Bacc(
        "TRN2", target_bir_lowering=False, debug=False, num_devices=N_CORES
    )
    x_d = nc.dram_tensor("x", [C, HW], F32, kind="ExternalInput")
    wqkvT_d = nc.dram_tensor("wqkvT", [C, 3 * C], F32R, kind="ExternalInput")
    wprojT_d = nc.dram_tensor("wprojT", [C, C], BF16, kind="ExternalInput")
    gsel_d = nc.dram_tensor("gsel", [C, C], F32, kind="ExternalInput")
    # per-channel vectors: [...,0]=gn_w [...,1]=gn_b [...,2]=qb/8 [...,3]=kb
    # [...,4]=proj_b + proj_w @ v_bias
    vecs_d = nc.dram_tensor("vecs", [CT, 128, 5], F32, kind="ExternalInput")
    vmask_d = nc.dram_tensor("vmask", [128, 2, 4 * 128], FP8, kind="ExternalInput")
    bsel_d = nc.dram_tensor("bsel", [128, 128], F32R, kind="ExternalInput")
    out_d = nc.dram_tensor("out", [C, HW], F32, kind="ExternalOutput")

    with ExitStack() as ctx:
        tc = ctx.enter_context(tile.TileContext(nc))
        const = ctx.enter_context(tc.tile_pool(name="const", bufs=1))
        big = ctx.enter_context(tc.tile_pool(name="big", bufs=1))
        xin = ctx.enter_context(tc.tile_pool(name="xin", bufs=1))
        xno = ctx.enter_context(tc.tile_pool(name="xno", bufs=1))
        epool = ctx.enter_context(tc.tile_pool(name="epool", bufs=10))
        small = ctx.enter_context(tc.tile_pool(name="small", bufs=1))
        mmps = ctx.enter_context(
            tc.tile_pool(name="mmps", bufs=2, space=bass.MemorySpace.PSUM)
        )
        pvps = ctx.enter_context(
            tc.tile_pool(name="pvps", bufs=2, space=bass.MemorySpace.PSUM)
        )

        # ---- load inputs ----
        # x in PCH column chunks per 128-row tile so GN stats trail the DMA
        vecs_sb = []
        wqkvT_sb = []
        wprojT_sb = []
        gsel_sb = []
        x_sb = [[None] * len(XCH) for _ in range(CT)]
        # x chunk-major first: GroupNorm stats trail this stream directly
        for cx, (po, pl) in enumerate(XCH):
            for ct in range(CT):
                xt = xin.tile([128, pl], F32, tag=f"x{ct}_{cx}", name=f"x{ct}_{cx}")
                nc.sync.dma_start(xt[:], x_d[ct * 128 : (ct + 1) * 128, po : po + pl])
                x_sb[ct][cx] = xt
        for ct in range(CT):
            vt = const.tile([128, 5], F32, tag=f"vecs{ct}")
            nc.sync.dma_start(vt[:], vecs_d[ct])
            vecs_sb.append(vt)
            gs = const.tile([128, C], F32, tag=f"gsel{ct}")
            nc.sync.dma_start(gs[:], gsel_d[ct * 128 : (ct + 1) * 128, :])
            gsel_sb.append(gs)
        for ct in range(CT):
            wq = const.tile([128, 3 * C], F32R, tag=f"wqkv{ct}")
            nc.sync.dma_start(wq[:], wqkvT_d[ct * 128 : (ct + 1) * 128, :])
            wqkvT_sb.append(wq)
        for ct in range(CT):
            wp = const.tile([128, C], BF16, tag=f"wproj{ct}")
            nc.sync.dma_start(wp[:], wprojT_d[ct * 128 : (ct + 1) * 128, :])
            wprojT_sb.append(wp)
        vmask_sb = const.tile([128, 2, 4 * 128], FP8, tag="vmask")
        nc.sync.dma_start(vmask_sb[:], vmask_d[:])
        # bsel rows {0,1} broadcast the two 1/denominator rows over each
        # head's 64 rows within a pair's 128-row block (K=2 matmul)
        bsel_sb = const.tile([128, 128], F32R, tag="bsel")
        nc.sync.dma_start(bsel_sb[:], bsel_d[:])

        # ---- ScalarE activation-table preload (dummy ops): the Exp and
        # Sqrt table loads cost ~1.3us each; issue them under the input DMA
        # instead of on the critical path ----
        tldr = small.tile([128, 1], F32, tag="tldr")
        nc.vector.memset(tldr[:], 1.0)
        nc.scalar.activation(tldr[:], tldr[:], AF.Identity)
        nc.scalar.activation(tldr[:], tldr[:], AF.Sqrt)
        nc.scalar.activation(tldr[:], tldr[:], AF.Exp)

        # ---- GroupNorm statistics ----
        # per-channel mean/var via bn_stats (9 subgroups of 256, chunked with
        # the x DMA), then average groups of 8 channels with the gsel matmul
        # on [mean, E[x^2]].  Emission is chunk-major so both channel tiles'
        # stats trail the DMA stream.
        st_sb = [
            small.tile([128, 6, 6], F32, tag=f"bnst{ct}", name=f"bnst{ct}")
            for ct in range(CT)
        ]
        for cx, (po, pl) in enumerate(XCH):
            for ct in range(CT):
                xr = x_sb[ct][cx].rearrange("p (n f) -> p n f", f=384)
                for k in range(2):
                    nc.vector.bn_stats(st_sb[ct][:, 2 * cx + k, :], xr[:, k, :])
        stats2 = []
        for ct in range(CT):
            mv = small.tile([128, 2], F32, tag=f"mv{ct}")
            nc.vector.bn_aggr(mv[:], st_sb[ct][:])
            s2 = small.tile([128, 2], F32, tag=f"s2{ct}")
            nc.vector.tensor_copy(s2[:, 0:1], mv[:, 0:1])
            nc.vector.tensor_tensor(s2[:, 1:2], mv[:, 0:1], mv[:, 0:1], op=OP.mult)
            nc.vector.tensor_add(s2[:, 1:2], s2[:, 1:2], mv[:, 1:2])
            stats2.append(s2)

        eps_sb = small.tile([128, 1], F32, tag="eps")
        nc.vector.memset(eps_sb[:], EPS)
        # 1/denominator: reciprocal lands in rscr rows {0,64} (legal DVE
        # write bases), then two tiny SBUF->SBUF DMAs pack them into rsum
        # rows {0,1} for the K=2 broadcast matmul (rows reused across pairs)
        rsum = small.tile([128, HW], F32R, tag="rsum")
        rscr = small.tile([128, 1024], F32, tag="rscr")
        dens = small.tile([128, 1024], F32, tag="dens")
        A_sb = []
        B_sb = []
        for mb in range(CT):
            # gsel is block-diagonal in 8-channel groups, so channel tile mb
            # only needs its own stats — no cross-tile accumulation
            ps = mmps.tile([128, 2], F32, tag=("stA" if mb % 2 == 0 else "stB"), bufs=1)
            nc.tensor.matmul(
                ps[:],
                gsel_sb[mb][:, mb * 128 : (mb + 1) * 128],
                stats2[mb][:],
                start=True,
                stop=True,
            )
            rstd = small.tile([128, 1], F32, tag=f"rstd{mb}")
            msq = small.tile([128, 1], F32, tag=f"msq{mb}")
            mg = small.tile([128, 1], F32, tag=f"mg{mb}")
            nc.vector.tensor_copy(mg[:], ps[:, 0:1])
            nc.vector.tensor_tensor(msq[:], mg[:], mg[:], op=OP.mult)
            nc.vector.tensor_tensor(rstd[:], ps[:, 1:2], msq[:], op=OP.subtract)
            # rstd = 1/sqrt(var + eps)
            nc.scalar.activation(rstd[:], rstd[:], AF.Sqrt, bias=eps_sb[:])
            nc.vector.reciprocal(rstd[:], rstd[:])
            a = small.tile([128, 1], F32, tag=f"A{mb}")
            b = small.tile([128, 1], F32, tag=f"B{mb}")
            nc.vector.tensor_tensor(a[:], vecs_sb[mb][:, 0:1], rstd[:], op=OP.mult)
            nc.vector.tensor_tensor(b[:], mg[:], a[:], op=OP.mult)
            nc.vector.tensor_tensor(b[:], vecs_sb[mb][:, 1:2], b[:], op=OP.subtract)
            A_sb.append(a)
            B_sb.append(b)

        # xn per (ct, chunk); residual base ot = x + eff_proj_bias on ScalarE
        # (idle at this point) per (ct, chunk)
        def x_cols(ct, lo, ln):
            "source APs for x columns [lo, lo+ln) out of the 768-wide chunks"
            out = []
            while ln > 0:
                cx = lo // 768
                off = lo - 768 * cx
                take = min(ln, 768 - off)
                out.append((x_sb[ct][cx][:, off : off + take], lo, take))
                lo += take
                ln -= take
            return out

        xn_sb = [[None] * len(PCH) for _ in range(CT)]
        for ci in range(len(PCH)):
            po, pl = PCH[ci]
            for ct in range(CT):
                xn = xno.tile([128, pl], F32R, tag=f"xn{ct}_{ci}", name=f"xn{ct}_{ci}")
                for src_ap, lo, take in x_cols(ct, po, pl):
                    dst = xn[:, lo - po : lo - po + take]
                    if (2 * ci + ct) % 2 == 0:
                        nc.vector.tensor_scalar(
                            dst, src_ap, A_sb[ct][:], B_sb[ct][:],
                            op0=OP.mult, op1=OP.add,
                        )
                    else:
                        nc.scalar.activation(
                            dst, src_ap, AF.Identity,
                            bias=B_sb[ct][:], scale=A_sb[ct][:],
                        )
                xn_sb[ct][ci] = xn
        # residual base ot = x + eff_proj_bias, one tile per ct (consumed only
        # in pair 1, so coarse write tracking is harmless)
        ot_sb = []
        for ct in range(CT):
            ot = xno.tile([128, HW], F32, tag=f"ot{ct}", name=f"ot{ct}")
            for cx, (po, pl) in enumerate(XCH):
                nc.scalar.activation(
                    ot[:, po : po + pl], x_sb[ct][cx][:], AF.Identity,
                    bias=vecs_sb[ct][:, 4:5],
                )
            ot_sb.append(ot)

        def xn_cols(ct, lo, ln):
            """AP for xn columns [lo, lo+ln) — must lie within one chunk."""
            ci = lo // 512
            return xn_sb[ct][ci][:, lo - 512 * ci : lo - 512 * ci + ln]

        # ---- qkv: q,k bf16 in channel-layout [128, HW] per head PAIR ----
        # chunk-major so the first matmuls start as soon as xn chunk 0 exists
        q_sb = [
            big.tile([128, HW], BF16, tag=f"q{p}", name=f"q{p}") for p in range(CT)
        ]
        k_sb = [
            big.tile([128, HW], BF16, tag=f"k{p}", name=f"k{p}") for p in range(CT)
        ]
        for ip, (po, pl) in enumerate(PCH):
            for which in range(2):  # 0 -> q, 1 -> k
                woff = which * C
                for mb in range(CT):
                    ps = mmps.tile(
                        [128, 1024], F32,
                        tag=("stA" if (ip * 2 + which) % 2 == 0 else "stB"), bufs=1,
                    )
                    for kt in range(CT):
                        nc.tensor.matmul(
                            ps[:, :pl],
                            wqkvT_sb[kt][
                                :, woff + mb * 128 : woff + (mb + 1) * 128
                            ],
                            xn_sb[kt][ip][:],
                            start=(kt == 0),
                            stop=(kt == CT - 1),
                        )
                    if which == 0:
                        # q = (q_raw + qb) / 8  (qb/8 is precomputed on host)
                        nc.vector.tensor_scalar(
                            q_sb[mb][:, po : po + pl],
                            ps[:, :pl],
                            0.125,
                            vecs_sb[mb][:, 2:3],
                            op0=OP.mult,
                            op1=OP.add,
                        )
                    else:
                        nc.vector.tensor_scalar(
                            k_sb[mb][:, po : po + pl],
                            ps[:, :pl],
                            vecs_sb[mb][:, 3:4],
                            None,
                            op0=OP.add,
                        )

        # ---- attention: one head PAIR at a time, pb-PAIR groups. The two
        # heads' K=64 score matmuls go to distinct PE row strips (base
        # partitions 0/64); a group's two score matmuls share one [128, 1024]
        # PSUM tile so ONE exp instruction covers both pb of the group.
        # vT tiles (fp8, with per-head ones column from vmask) are produced
        # just-in-time during pair 0's first range. ----
        attn_sb = [
            big.tile([128, HW], BF16, tag=f"attn{p}", name=f"attn{p}")
            for p in range(CT)
        ]
        vt2 = []
        _tagc = [0]

        def next_tag():
            _tagc[0] += 1
            return ("stA", "stB", "stC")[_tagc[0] % 3]

        def produce_vt2(pbp):
            vt = big.tile([128, 2, 4 * 128], FP8, tag=f"vt{pbp}", name=f"vt{pbp}")
            nc.sync.dma_start(vt[:], vmask_sb[:])
            for par in range(2):
                pb = 2 * pbp + par
                ps = mmps.tile([128, 1024], F32, tag=next_tag(), bufs=1, name="vps")
                for kt in range(CT):
                    nc.tensor.matmul(
                        ps[:, :C],
                        xn_cols(kt, pb * 128, 128),
                        wqkvT_sb[kt][:, 2 * C : 3 * C],
                        start=(kt == 0),
                        stop=(kt == CT - 1),
                    )
                dst = vt[:, par, :].rearrange("p (h c) -> p h c", h=NH)[:, :, 0:HD]
                src = ps[:, :C].rearrange("p (h c) -> p h c", c=HD)
                nc.vector.tensor_copy(dst, src)
            vt2.append(vt)

        def emit_epilogue(p, io, il, pvs):
            """Broadcast 1/denominator over each head's 64 rows (two
            concurrent K=1 matmuls on disjoint PE tiles), normalize the attn
            chunk, and for pair 1 stream proj + residual + output DMA.
            Deferred past the next irange's first score groups so the PE
            queue never stalls waiting on the DVE reciprocal."""
            rs = mmps.tile([128, 1024], F32, tag=next_tag(), bufs=1, name="rs")
            nc.tensor.matmul(
                rs[:, :il],
                bsel_sb[0:2, :],
                rsum[0:2, io : io + il],
                start=True,
                stop=True,
            )
            nc.vector.tensor_tensor(
                attn_sb[p][:, io : io + il],
                attn_sb[p][:, io : io + il],
                rs[:, :il],
                op=OP.mult,
            )
            if p == CT - 1:
                for ct in range(CT):
                    ps = mmps.tile(
                        [128, 1024], F32, tag=next_tag(), bufs=1, name="prj"
                    )
                    for kt in range(CT):
                        nc.tensor.matmul(
                            ps[:, :il],
                            wprojT_sb[kt][:, ct * 128 : (ct + 1) * 128],
                            attn_sb[kt][:, io : io + il],
                            start=(kt == 0),
                            stop=(kt == CT - 1),
                        )
                    nc.vector.tensor_tensor(
                        ot_sb[ct][:, io : io + il],
                        ot_sb[ct][:, io : io + il],
                        ps[:, :il],
                        op=OP.add,
                    )
                    nc.sync.dma_start(
                        out_d[ct * 128 : (ct + 1) * 128, io : io + il],
                        ot_sb[ct][:, io : io + il],
                    )

        epi_pending = []
        for p in range(CT):
            for ri, (io, il) in enumerate(IRANGES):
                es = {}
                pvs = {}
                for hh in range(2):
                    pvs[hh] = pvps.tile(
                        [128, 512], F32, tag="pvps", name=f"pv{p}_{io}_{hh}"
                    )

                def emit_pv(pbp, p=p, io=io, il=il, es=es, pvs=pvs):
                    for hh in range(2):
                        h = 2 * p + hh
                        nc.tensor.matmul(
                            pvs[hh][:, :il],
                            vt2[pbp][:, :, h * 128 : (h + 1) * 128],
                            es[(hh, pbp)][:, :, :il],
                            start=(pbp == 0),
                            stop=(pbp == PBP - 1),
                            perf_mode=mybir.MatmulPerfMode.DoubleRow,
                        )

                for pbp in range(PBP):
                    if p == 0 and ri == 0:
                        produce_vt2(pbp)
                    pat = EXP_PAT[pbp % 5]
                    sts = {}
                    for hh in range(2):
                        sts[hh] = mmps.tile(
                            [128, 1024], F32, tag=next_tag(), bufs=1,
                            name=f"st{hh}",
                        )
                    for par in range(2):
                        pb = 2 * pbp + par
                        for hh in range(2):
                            ro = hh * 64
                            nc.tensor.matmul(
                                sts[hh][:, par * il : par * il + il],
                                k_sb[p][ro : ro + 64, pb * 128 : (pb + 1) * 128],
                                q_sb[p][ro : ro + 64, io : io + il],
                                start=True,
                                stop=True,
                            )
                    for hh in range(2):
                        es[(hh, pbp)] = epool.tile(
                            [128, 2, il], FP8, tag="E",
                            name=f"e{p}_{io}_{hh}_{pbp}",
                        )
                        e = es[(hh, pbp)]
                        src = sts[hh][:, : 2 * il].rearrange(
                            "p (a b) -> p a b", a=2
                        )
                        if pat[hh] == "V":
                            nc.vector.tensor_scalar(
                                e[:].bitcast(I8),
                                src,
                                A8,
                                C8,
                                op0=OP.mult,
                                op1=OP.add,
                            )
                        else:
                            nc.scalar.activation(e[:], src, AF.Exp)
                    if pbp == 3 and epi_pending:
                        emit_epilogue(*epi_pending.pop())
                    if pbp - LA >= 0:
                        emit_pv(pbp - LA)
                for pbp in range(PBP - LA, PBP):
                    emit_pv(pbp)

                # attn rows out of PSUM (ScalarE) + denominator reciprocals
                # (DVE) issue immediately; the tensor-engine part of the
                # epilogue is deferred into the next irange
                for hh in range(2):
                    srow = ONES_COL[hh]
                    co = 512 * hh
                    # Identity (not Copy) so the ScalarE table set during
                    # attention stays {Exp, Identity} -> no table reloads
                    nc.scalar.activation(
                        attn_sb[p][hh * 64 : (hh + 1) * 64, io : io + il],
                        pvs[hh][0:64, :il],
                        AF.Identity,
                    )
                    # PSUM denominator row -> SBUF row 0 on ScalarE, then
                    # reciprocal SBUF->SBUF on DVE.  Both at partition base 0:
                    # the custom-DVE op silently writes zeros at base 64 and
                    # misreads PSUM at base 64/96 on hardware.
                    nc.scalar.activation(
                        dens[0:1, co : co + il],
                        pvs[hh][srow : srow + 1, :il],
                        AF.Identity,
                    )
                    nc.vector.reciprocal_approx_fast(
                        rscr[0:1, co : co + il],
                        dens[0:1, co : co + il],
                    )
                for hh in range(2):
                    nc.sync.dma_start(
                        rsum[hh : hh + 1, io : io + il],
                        rscr[0:1, 512 * hh : 512 * hh + il].bitcast(F32R),
                    )
                epi_pending.append((p, io, il, pvs))

        while epi_pending:
            emit_epilogue(*epi_pending.pop())

    nc.compile()
    return nc


_NC = None


def _get_nc():
    global _NC
    if _NC is None:
        _NC = _build()
    return _NC


def _host_prep(x, gn_w, gn_b, qkv_w, qkv_b, proj_w, proj_b):
    import ml_dtypes

    f32 = np.float32
    x = np.asarray(x, dtype=f32)
    gn_w = np.asarray(gn_w, dtype=f32)
    gn_b = np.asarray(gn_b, dtype=f32)
    qkv_w = np.asarray(qkv_w, dtype=f32)
    qkv_b = np.asarray(qkv_b, dtype=f32)
    proj_w = np.asarray(proj_w, dtype=f32)
    proj_b = np.asarray(proj_b, dtype=f32)

    b = x.shape[0]
    xs = np.ascontiguousarray(x.reshape(b, C, HW))

    wqkvT = np.ascontiguousarray(qkv_w.T)
    wprojT = np.ascontiguousarray(proj_w.T).astype(ml_dtypes.bfloat16)

    gsel = np.zeros((C, C), dtype=f32)
    for g in range(G):
        gsel[g * 8 : (g + 1) * 8, g * 8 : (g + 1) * 8] = 1.0 / 8.0

    bsel = np.zeros((128, 128), dtype=f32)
    bsel[0, 0:64] = 1.0
    bsel[1, 64:128] = 1.0

    pbeff = proj_b + proj_w @ qkv_b[2 * C : 3 * C]
    vecs = np.stack(
        [gn_w, gn_b, qkv_b[:C] / 8.0, qkv_b[C : 2 * C], pbeff], axis=-1
    ).reshape(CT, 128, 5)
    vecs = np.ascontiguousarray(vecs.astype(f32))

    vmask = np.zeros((128, 2, 4 * 128), dtype=np.float32)
    for h in range(NH):
        vmask[:, :, h * 128 + ONES_COL[h % 2]] = 1.0
    vmask = vmask.astype(ml_dtypes.float8_e4m3)

    shared = {
        "wqkvT": wqkvT,
        "wprojT": wprojT,
        "gsel": gsel,
        "bsel": bsel,
        "vecs": vecs,
        "vmask": vmask,
    }
    in_maps = [dict(shared, x=np.ascontiguousarray(xs[i])) for i in range(b)]
    return in_maps, x.shape


def _run(inputs, **run_kwargs):
    nc = _get_nc()
    in_maps, xshape = _host_prep(**inputs)
    res = run_bass_kernel_spmd(
        nc, in_maps, core_ids=list(range(N_CORES)), **run_kwargs
    )
    out = np.stack([res.results[i]["out"] for i in range(N_CORES)])
    return out.reshape(xshape).astype(np.float32), res


def kernel(**inputs):
    out, _ = _run(inputs)
    return out
